# revision 14
# baseline (speedup 1.0000x reference)
"""Trainium2 Bass kernel for CustomMultiHeadAttention (v2: fp8 + exp split).

Problem: x[2,2048,1024], 16 heads, Dh=64. y = MHA(x) with Q/K/V/O projections.

Sharding (8 cores, no collectives): core c -> batch b = c//4, head-quarter
hq = c%4 (4 heads, 256 model cols). Each core computes Q,K,V for its 4 heads
over the full sequence of its batch, attention, and a PARTIAL o_proj (its 256
rows of Wo). Host sums the 4 partials per batch and adds bo + bv@Wo (bv folds
out since softmax rows sum to 1; bk also folds out: it shifts every score of a
query column equally, which softmax cancels).

Numerics / engine split:
  - Q/K/V projections run in fp8e4 (x and W quantized, W pre-scaled by 32 to
    clear the e4m3 denormal band; the 1/32 is folded into the PSUM->SBUF
    copy). DoubleRow packs ko-pairs: out = sum_j lhsT[:,j,:].T @ rhs[:,j,:].
  - QK^T runs in bf16 (fp8 without DoubleRow has no speed benefit and QK
    accuracy matters most).
  - Softmax has no max-subtraction; instead exp(0.125*s - 3) keeps P in
    e4m3 range ([6e-6, ~12] for |0.125 s| <= ~5.5); the global e^-3 factor
    cancels in the sumexp normalization.
  - exp is split across engines: most tiles on ACT (table exp, fp8 out),
    a subset on DVE as a Schraudolph fast-exp: uint8 = 1.4427*s + B directly
    builds the e4m3 bit pattern (B centers the linear-mantissa error; f32->u8
    conversion saturates negatives to 0 = fp8 zero for underflow).
  - AV runs fp8 DoubleRow over key-tile pairs; V carries a ones column so
    row 64 of O'^T is sumexp. 1/sumexp (DVE reciprocal) is partition-broadcast
    on GPSIMD, normalize on DVE, o_proj staging copies on ACT, y out in bf16.
"""

import numpy as np
import ml_dtypes

import concourse.mybir as mybir
import concourse.tile as tile
from concourse import bacc
from concourse.bass_utils import run_bass_kernel_spmd

P = 128
S = 2048
D = 1024
H = 16
DH = 64
HPC = 4          # heads per core
HD = HPC * DH    # 256 model cols per core
KO = D // P      # 8 contraction subtiles
KT_N = S // P    # 16 key tiles
QT = 512
QT_N = S // QT
N_CORES = 8
VP = 80          # V inner stride: d 0..63, ones col 64, pad to 80 (16B align)

FP8_PROJ = False
FP8_AV = False
WSCALE = 32.0 if FP8_PROJ else 1.0
# (step, h2) pairs whose exp runs on DVE (fast-exp); rest on ACT.
USE_GPSIMD_BCAST = False
DVE_SET_EVEN = {(s, 1) for s in range(6)}
DVE_SET_ODD = DVE_SET_EVEN
A_DVE = 23.083120654230615          # 16*log2(e): bits16 = A*s + B
B_DVE = 15696.4                     # 128*(127 - 3*log2e) - 5.5 centering

BF16 = mybir.dt.bfloat16
F32 = mybir.dt.float32
U8 = mybir.dt.uint8
I16 = mybir.dt.int16
FP8 = mybir.dt.float8e4
EXP = mybir.ActivationFunctionType.Exp
COPY = mybir.ActivationFunctionType.Copy
MUL = mybir.AluOpType.mult
ADD = mybir.AluOpType.add
DR = mybir.MatmulPerfMode.DoubleRow

PD = FP8 if FP8_PROJ else BF16
AVD = FP8 if FP8_AV else BF16
ETD = U8 if FP8_AV else BF16

_CACHE = {}


def _build_program():
    nc = bacc.Bacc(
        "TRN2",
        target_bir_lowering=False,
        debug=False,
        enable_asserts=False,
        num_devices=N_CORES,
    )
    xT = nc.dram_tensor("xT", [P, KO, S], PD, kind="ExternalInput").ap()
    wq = nc.dram_tensor("wq", [P, KO, HD], PD, kind="ExternalInput").ap()
    wk = nc.dram_tensor("wk", [P, KO, HD], PD, kind="ExternalInput").ap()
    wv = nc.dram_tensor("wv", [P, KO, HD], PD, kind="ExternalInput").ap()
    wo = nc.dram_tensor("wo", [P, HD // P, D], BF16, kind="ExternalInput").ap()
    bq = nc.dram_tensor("bq", [P, HD // P], F32, kind="ExternalInput").ap()
    y = nc.dram_tensor("y", [S, D], BF16, kind="ExternalOutput").ap()

    with tile.TileContext(nc) as tc:
        _body(tc, y, xT, wq, wk, wv, wo, bq)
    nc.compile()
    return nc


def _mm_proj(nc, out, w_or_x_lhsT, rhs_sb, lhs_cols, rhs_cols):
    """Contraction over KO subtiles: DoubleRow ko-pairs in fp8, else 8x bf16.
    lhsT/rhs are [P, KO, *] SBUF tiles; cols are slices into the last dim."""
    if FP8_PROJ:
        for t in range(KO // 2):
            nc.tensor.matmul(
                out,
                lhsT=w_or_x_lhsT[:, 2 * t : 2 * t + 2, lhs_cols],
                rhs=rhs_sb[:, 2 * t : 2 * t + 2, rhs_cols],
                start=(t == 0),
                stop=(t == KO // 2 - 1),
                perf_mode=DR,
            )
    else:
        for ko in range(KO):
            nc.tensor.matmul(
                out,
                lhsT=w_or_x_lhsT[:, ko, lhs_cols],
                rhs=rhs_sb[:, ko, rhs_cols],
                start=(ko == 0),
                stop=(ko == KO - 1),
            )


def _body(tc, y, xT, wq, wk, wv, wo, bq):
    nc = tc.nc
    with (
        tc.tile_pool(name="const", bufs=1) as const,
        tc.tile_pool(name="big", bufs=1) as big,
        tc.tile_pool(name="work", bufs=3) as work,
        tc.tile_pool(name="exps", bufs=48) as exps,
        tc.tile_pool(name="yst", bufs=2) as yst,
        tc.tile_pool(name="psw", bufs=2, space="PSUM") as psw,  # [P,2,QT]
        tc.tile_pool(name="psk", bufs=2, space="PSUM") as psk,  # [P,QT]
        tc.tile_pool(name="psa", bufs=2, space="PSUM") as psa,  # [P,QT]
    ):
        # ---- PE warmup on a gpsimd-zeroed tile (DVE stays free for real
        # work); spans the ~3us p-state ramp while the first DMAs land ----
        wu = const.tile([P, QT], BF16, tag="wu")
        nc.gpsimd.memset(wu[:], 0.0)
        pwu = psk.tile([P, QT], F32, tag="psk", name="pwu")
        for _ in range(12):
            nc.tensor.matmul(pwu[:], lhsT=wu[:, 0:P], rhs=wu[:], start=True,
                             stop=True)

        # ---- DMA order tuned so the K/Q projections can start early ----
        wk_sb = const.tile([P, KO, HD], PD, tag="wk")
        nc.sync.dma_start(wk_sb[:], wk)
        xT_sb = big.tile([P, KO, S], PD, tag="xT")
        nc.sync.dma_start(xT_sb[:, :, 0:QT], xT[:, :, 0:QT])
        wq_sb = const.tile([P, KO, HD], PD, tag="wq")
        nc.sync.dma_start(wq_sb[:], wq)
        bq_sb = const.tile([P, HD // P], F32, tag="bq")
        nc.sync.dma_start(bq_sb[:], bq)
        for c in range(1, QT_N):
            cs = slice(c * QT, (c + 1) * QT)
            nc.sync.dma_start(xT_sb[:, :, cs], xT[:, :, cs])
        wv_sb = const.tile([P, KO, HD], PD, tag="wv")
        nc.sync.dma_start(wv_sb[:], wv)
        wo_sb = const.tile([P, HD // P, D], BF16, tag="wo")
        nc.sync.dma_start(wo_sb[:], wo)

        bias3 = const.tile([P, 1], F32, tag="bias3")
        nc.vector.memset(bias3[:], -3.0)
        # dummy 1-element exp: pulls the ~2.7us ACT table load into the
        # initial DMA wait instead of the first real softmax tile
        dume = const.tile([P, 1], BF16, tag="dume")
        nc.scalar.activation(dume[:], bias3[:], EXP)

        # V with ones column at index DH (-> sumexp row of O'^T)
        v_sb = big.tile([P, KT_N // 2, HPC, 2, VP], AVD, tag="v")
        nc.vector.memset(v_sb[:, :, :, :, DH : DH + 2], 1.0)

        qT_sb = big.tile([P, HD // P, S], BF16, tag="qT")
        kT_sb = big.tile([P, HD // P, S], BF16, tag="kT")
        oT_sb = big.tile([P, HD // P, S], BF16, tag="oT")
        rbs = [const.tile([P, QT], BF16, tag=f"rb{i}", name=f"rb{i}")
               for i in range(2)]
        for t in rbs:
            # rows 65..127 are read (x0) by the pb broadcast matmul; they
            # must not hold NaN bit patterns from uninitialized SBUF
            nc.vector.memset(t[:], 0.0)
        if not USE_GPSIMD_BCAST:
            # selector for the PE partition-broadcast matmul: lhsT row 64
            # (= partition 64, all-ones) x rhs row 64 (1/sumexp) -> pb[0:64]
            sel0 = const.tile([P, DH], BF16, tag="sel0")
            nc.vector.memset(sel0[:], 0.0)
            nc.vector.memset(sel0[DH : DH + 1, :], 1.0)

        def k_proj(mt, c):
            cs = slice(c * QT, (c + 1) * QT)
            pk = psk.tile([P, QT], F32, tag="psk", name="pk")
            _mm_proj(nc, pk[:], wk_sb, xT_sb, slice(mt * P, (mt + 1) * P), cs)
            # bk cancels in softmax; only the 1/WSCALE rescale is applied
            nc.vector.tensor_scalar_mul(kT_sb[:, mt, cs], pk[:], 1.0 / WSCALE)

        def q_proj(qt, mt):
            qs = slice(qt * QT, (qt + 1) * QT)
            pq = psk.tile([P, QT], F32, tag="psk", name="pq")
            _mm_proj(nc, pq[:], wq_sb, xT_sb, slice(mt * P, (mt + 1) * P), qs)
            nc.vector.tensor_scalar(
                qT_sb[:, mt, qs], pq[:], 1.0 / WSCALE, bq_sb[:, mt : mt + 1],
                MUL, ADD,
            )

        def v_proj(st):
            ss = slice(st * P, (st + 1) * P)
            pv = psk.tile([P, QT], F32, tag="psk", name="pv")
            _mm_proj(nc, pv[:, 0:HD], xT_sb, wv_sb, ss, slice(0, HD))
            nc.vector.tensor_scalar_mul(
                v_sb[:, st // 2, :, st % 2, 0:DH],
                pv[:, 0:HD].rearrange("p (h c) -> p h c", h=HPC),
                1.0 / WSCALE,
            )

        def qk_phase(qt, hp, tiles=None, steps=None):
            qs = slice(qt * QT, (qt + 1) * QT)
            dve_set = DVE_SET_EVEN if hp == 0 else DVE_SET_ODD
            if tiles is None:
                tiles = [[None, None] for _ in range(KT_N // 2)]
            for step in steps if steps is not None else range(KT_N // 2):
                kt = 2 * step
                for h2 in range(2):
                    pr = slice(h2 * DH, (h2 + 1) * DH)
                    pqk = psw.tile([P, 2, QT], F32, tag="psw", name="pqk")
                    for j in range(2):
                        nc.tensor.matmul(
                            pqk[:, j, :],
                            lhsT=kT_sb[pr, hp, (kt + j) * P : (kt + j + 1) * P],
                            rhs=qT_sb[pr, hp, qs],
                            start=True,
                            stop=True,
                        )
                    et = exps.tile([P, 2, QT], ETD, tag="exps",
                                   name=f"e{step}{h2}")
                    if (step, h2) in dve_set:
                        nc.vector.tensor_scalar(
                            et[:].bitcast(I16) if not FP8_AV else et[:],
                            pqk[:], A_DVE, B_DVE, MUL, ADD)
                    elif FP8_AV:
                        nc.scalar.activation(
                            et[:].bitcast(FP8), pqk[:], EXP,
                            scale=0.125, bias=bias3[:, 0:1])
                    else:
                        nc.scalar.activation(et[:], pqk[:], EXP,
                                             scale=0.125, bias=bias3[:, 0:1])
                    tiles[step][h2] = et
            return tiles

        def av_phase(qt, hp, tiles, h2_order=(0, 1)):
            qs = slice(qt * QT, (qt + 1) * QT)
            for h2 in h2_order:
                h = 2 * hp + h2
                po = psa.tile([P, QT], F32, tag="psa", name="po")
                if FP8_AV:
                    for t in range(KT_N // 2):
                        nc.tensor.matmul(
                            po[0 : DH + 2, :],
                            lhsT=v_sb[:, t, h, :, 0 : DH + 2],
                            rhs=tiles[t][h2][:].bitcast(FP8),
                            start=(t == 0),
                            stop=(t == KT_N // 2 - 1),
                            perf_mode=DR,
                        )
                else:
                    for kt in range(KT_N):
                        nc.tensor.matmul(
                            po[0 : DH + 1, :],
                            lhsT=v_sb[:, kt // 2, h, kt % 2, 0 : DH + 1],
                            rhs=tiles[kt // 2][h2][:, kt % 2, :],
                            start=(kt == 0),
                            stop=(kt == KT_N - 1),
                        )
                rb = rbs[(2 * qt + hp) % 2]
                with nc.allow_low_precision(
                    reason="bf16 1/sumexp costs ~0.2% rel err, within budget"
                ):
                    nc.vector.reciprocal(rb[DH : DH + 1, :], po[DH : DH + 1, :])
                if USE_GPSIMD_BCAST:
                    bc = work.tile([DH, QT], BF16, tag="bc", name="bc")
                    nc.gpsimd.partition_broadcast(bc[:], rb[DH:P, :])
                    num = po[0:DH, :]
                else:
                    bcp = psa.tile([P, QT], F32, tag="psa", name="bcp")
                    nc.tensor.matmul(bcp[0:DH, :], lhsT=sel0[DH:P, :],
                                     rhs=rb[DH:P, :], start=True, stop=True)
                    bc = bcp[0:DH, :]
                    pon = work.tile([DH, QT], F32, tag="pon", name="pon")
                    nc.vector.tensor_copy(out=pon[:], in_=po[0:DH, :])
                    num = pon[:]
                if h2 == 0:
                    nc.vector.tensor_tensor(
                        oT_sb[0:DH, hp, qs], num, bc, MUL)
                else:
                    # odd head rows belong on partitions 64..127; DVE is
                    # lane-locked, so stage bf16 and lane-shift via DMA
                    sh = work.tile([DH, QT], BF16, tag="sh", name="sh")
                    nc.vector.tensor_tensor(sh[:], num, bc, MUL)
                    nc.sync.dma_start(oT_sb[DH:P, hp, qs], sh[:])

        def o_proj(qt, sts=None, on_dve=False):
            for st in sts if sts is not None else range(QT // P):
                rows = slice(qt * QT + st * P, qt * QT + (st + 1) * P)
                yt = yst.tile([P, D], BF16, tag="yt", name="yt")
                for nt2 in range(D // QT):
                    py = psk.tile([P, QT], F32, tag="psk", name="py")
                    for ks in range(HD // P):
                        nc.tensor.matmul(
                            py[:],
                            lhsT=oT_sb[:, ks, rows],
                            rhs=wo_sb[:, ks, nt2 * QT : (nt2 + 1) * QT],
                            start=(ks == 0),
                            stop=(ks == HD // P - 1),
                        )
                    # staging copy on ACT in steady state (frees DVE for
                    # fast-exp); on DVE for the tail (ACT chews last exps)
                    if on_dve:
                        nc.vector.tensor_copy(
                            out=yt[:, nt2 * QT : (nt2 + 1) * QT], in_=py[:])
                    else:
                        nc.scalar.activation(
                            yt[:, nt2 * QT : (nt2 + 1) * QT], py[:], COPY)
                nc.sync.dma_start(y[rows, :], yt[:])

        # ---- emission schedule (order = scheduler priority) ----
        k_proj(0, 0)
        q_proj(0, 0)
        k_proj(0, 1)
        k_proj(0, 2)
        k_proj(0, 3)
        e = {(0, 0): qk_phase(0, 0)}
        k_proj(1, 0)
        k_proj(1, 1)
        k_proj(1, 2)
        k_proj(1, 3)
        q_proj(0, 1)
        e[(0, 1)] = qk_phase(0, 1)
        for st in range(KT_N):
            v_proj(st)
        q_proj(1, 0)
        q_proj(1, 1)
        for qt in range(1, QT_N):
            lastit = qt == QT_N - 1
            av_phase(qt - 1, 0, e[(qt - 1, 0)], h2_order=(0,))
            t0 = qk_phase(qt, 0, steps=range(0, 4))
            av_phase(qt - 1, 0, e.pop((qt - 1, 0)), h2_order=(1,))
            if lastit:
                t1 = qk_phase(qt, 1, steps=range(0, 4))
            qk_phase(qt, 0, tiles=t0, steps=range(4, 8))
            e[(qt, 0)] = t0
            av_phase(qt - 1, 1, e[(qt - 1, 1)], h2_order=(0,))
            if not lastit:
                t1 = qk_phase(qt, 1, steps=range(0, 4))
            av_phase(qt - 1, 1, e.pop((qt - 1, 1)), h2_order=(1,))
            if lastit:
                qk_phase(qt, 1, tiles=t1, steps=range(4, 8))
            o_proj(qt - 1, sts=(0, 1))
            if not lastit:
                qk_phase(qt, 1, tiles=t1, steps=range(4, 8))
            e[(qt, 1)] = t1
            o_proj(qt - 1, sts=(2, 3))
            if qt + 1 < QT_N:
                q_proj(qt + 1, 0)
                q_proj(qt + 1, 1)
        last = QT_N - 1
        av_phase(last, 0, e.pop((last, 0)))
        # odd head first: its lane-shift DMA (the last o_proj dependency)
        # overlaps the even head's AV + normalize
        av_phase(last, 1, e.pop((last, 1)), h2_order=(1, 0))
        o_proj(last, sts=(0, 1), on_dve=True)
        o_proj(last, sts=(2, 3))


def _prep_inputs(x, Wq, bq, Wk, bk, Wv, bv, Wo, bo):
    bf = ml_dtypes.bfloat16
    pd = ml_dtypes.float8_e4m3 if FP8_PROJ else bf
    x = np.asarray(x, np.float32)
    in_maps = []
    for c in range(N_CORES):
        b, hq = c // 4, c % 4
        cs = slice(hq * HD, (hq + 1) * HD)

        xTr = np.ascontiguousarray(
            x[b].T.reshape(KO, P, S).transpose(1, 0, 2)).astype(pd)

        def wprep(W):
            Wc = np.asarray(W, np.float32)[:, cs] * WSCALE
            return np.ascontiguousarray(
                Wc.reshape(KO, P, HD).transpose(1, 0, 2)).astype(pd)

        in_maps.append({
            "xT": xTr,
            "wq": wprep(Wq),
            "wk": wprep(Wk),
            "wv": wprep(Wv),
            "wo": np.ascontiguousarray(
                np.asarray(Wo, np.float32)[cs, :]
                .reshape(HD // P, P, D).transpose(1, 0, 2)).astype(bf),
            "bq": np.ascontiguousarray(
                np.asarray(bq, np.float32)[cs].reshape(HD // P, P).T),
        })
    return in_maps


def get_program():
    if "nc" not in _CACHE:
        _CACHE["nc"] = _build_program()
    return _CACHE["nc"]


def run(inputs, **kw):
    nc = get_program()
    in_maps = _prep_inputs(**inputs)
    res = run_bass_kernel_spmd(nc, in_maps, core_ids=list(range(N_CORES)), **kw)
    # final bias: bo + bv @ Wo (bv folds out of attention since softmax rows
    # sum to 1), computed in fp32 on host
    bias = np.asarray(inputs["bo"], np.float32) + np.asarray(
        inputs["bv"], np.float32
    ) @ np.asarray(inputs["Wo"], np.float32)
    out = np.empty((2, S, D), np.float32)
    for b in range(2):
        acc = res.results[4 * b]["y"].astype(np.float32)
        for i in range(1, 4):
            acc = acc + res.results[4 * b + i]["y"].astype(np.float32)
        out[b] = acc + bias
    return out, res


def kernel(**inputs):
    out, _ = run(inputs)
    return out


# revision 15
# speedup vs baseline: 1.0198x; 1.0198x over previous
"""Trainium2 Bass kernel for CustomMultiHeadAttention (v2: fp8 + exp split).

Problem: x[2,2048,1024], 16 heads, Dh=64. y = MHA(x) with Q/K/V/O projections.

Sharding (8 cores, no collectives): core c -> batch b = c//4, head-quarter
hq = c%4 (4 heads, 256 model cols). Each core computes Q,K,V for its 4 heads
over the full sequence of its batch, attention, and a PARTIAL o_proj (its 256
rows of Wo). Host sums the 4 partials per batch and adds bo + bv@Wo (bv folds
out since softmax rows sum to 1; bk also folds out: it shifts every score of a
query column equally, which softmax cancels).

Numerics / engine split:
  - Q/K/V projections run in fp8e4 (x and W quantized, W pre-scaled by 32 to
    clear the e4m3 denormal band; the 1/32 is folded into the PSUM->SBUF
    copy). DoubleRow packs ko-pairs: out = sum_j lhsT[:,j,:].T @ rhs[:,j,:].
  - QK^T runs in bf16 (fp8 without DoubleRow has no speed benefit and QK
    accuracy matters most).
  - Softmax has no max-subtraction; instead exp(0.125*s - 3) keeps P in
    e4m3 range ([6e-6, ~12] for |0.125 s| <= ~5.5); the global e^-3 factor
    cancels in the sumexp normalization.
  - exp is split across engines: most tiles on ACT (table exp, fp8 out),
    a subset on DVE as a Schraudolph fast-exp: uint8 = 1.4427*s + B directly
    builds the e4m3 bit pattern (B centers the linear-mantissa error; f32->u8
    conversion saturates negatives to 0 = fp8 zero for underflow).
  - AV runs fp8 DoubleRow over key-tile pairs; V carries a ones column so
    row 64 of O'^T is sumexp. 1/sumexp (DVE reciprocal) is partition-broadcast
    on GPSIMD, normalize on DVE, o_proj staging copies on ACT, y out in bf16.
"""

import numpy as np
import ml_dtypes

import concourse.mybir as mybir
import concourse.tile as tile
from concourse import bacc
from concourse.bass_utils import run_bass_kernel_spmd

P = 128
S = 2048
D = 1024
H = 16
DH = 64
HPC = 4          # heads per core
HD = HPC * DH    # 256 model cols per core
KO = D // P      # 8 contraction subtiles
KT_N = S // P    # 16 key tiles
QT = 512
QT_N = S // QT
N_CORES = 8
VP = 80          # V inner stride: d 0..63, ones col 64, pad to 80 (16B align)

FP8_PROJ = False
FP8_AV = False
WSCALE = 32.0 if FP8_PROJ else 1.0
# (step, h2) pairs whose exp runs on DVE (fast-exp); rest on ACT.
USE_GPSIMD_BCAST = False
DVE_SET_EVEN = {(1, 0), (1, 1), (3, 0), (3, 1), (5, 0), (5, 1)}
DVE_SET_ODD = DVE_SET_EVEN
A_DVE = 23.083120654230615          # 16*log2(e): bits16 = A*s + B
B_DVE = 15696.4                     # 128*(127 - 3*log2e) - 5.5 centering

BF16 = mybir.dt.bfloat16
F32 = mybir.dt.float32
U8 = mybir.dt.uint8
I16 = mybir.dt.int16
FP8 = mybir.dt.float8e4
EXP = mybir.ActivationFunctionType.Exp
COPY = mybir.ActivationFunctionType.Copy
MUL = mybir.AluOpType.mult
ADD = mybir.AluOpType.add
DR = mybir.MatmulPerfMode.DoubleRow

PD = FP8 if FP8_PROJ else BF16
AVD = FP8 if FP8_AV else BF16
ETD = U8 if FP8_AV else BF16

_CACHE = {}


def _build_program():
    nc = bacc.Bacc(
        "TRN2",
        target_bir_lowering=False,
        debug=False,
        enable_asserts=False,
        num_devices=N_CORES,
    )
    xT = nc.dram_tensor("xT", [P, KO, S], PD, kind="ExternalInput").ap()
    wq = nc.dram_tensor("wq", [P, KO, HD], PD, kind="ExternalInput").ap()
    wk = nc.dram_tensor("wk", [P, KO, HD], PD, kind="ExternalInput").ap()
    wv = nc.dram_tensor("wv", [P, KO, HD], PD, kind="ExternalInput").ap()
    wo = nc.dram_tensor("wo", [P, HD // P, D], BF16, kind="ExternalInput").ap()
    bq = nc.dram_tensor("bq", [P, HD // P], F32, kind="ExternalInput").ap()
    y = nc.dram_tensor("y", [S, D], BF16, kind="ExternalOutput").ap()

    with tile.TileContext(nc) as tc:
        _body(tc, y, xT, wq, wk, wv, wo, bq)
    nc.compile()
    return nc


def _mm_proj(nc, out, w_or_x_lhsT, rhs_sb, lhs_cols, rhs_cols):
    """Contraction over KO subtiles: DoubleRow ko-pairs in fp8, else 8x bf16.
    lhsT/rhs are [P, KO, *] SBUF tiles; cols are slices into the last dim."""
    if FP8_PROJ:
        for t in range(KO // 2):
            nc.tensor.matmul(
                out,
                lhsT=w_or_x_lhsT[:, 2 * t : 2 * t + 2, lhs_cols],
                rhs=rhs_sb[:, 2 * t : 2 * t + 2, rhs_cols],
                start=(t == 0),
                stop=(t == KO // 2 - 1),
                perf_mode=DR,
            )
    else:
        for ko in range(KO):
            nc.tensor.matmul(
                out,
                lhsT=w_or_x_lhsT[:, ko, lhs_cols],
                rhs=rhs_sb[:, ko, rhs_cols],
                start=(ko == 0),
                stop=(ko == KO - 1),
            )


def _body(tc, y, xT, wq, wk, wv, wo, bq):
    nc = tc.nc
    with (
        tc.tile_pool(name="const", bufs=1) as const,
        tc.tile_pool(name="big", bufs=1) as big,
        tc.tile_pool(name="work", bufs=3) as work,
        tc.tile_pool(name="exps", bufs=48) as exps,
        tc.tile_pool(name="yst", bufs=2) as yst,
        tc.tile_pool(name="psw", bufs=2, space="PSUM") as psw,  # [P,2,QT]
        tc.tile_pool(name="psk", bufs=2, space="PSUM") as psk,  # [P,QT]
        tc.tile_pool(name="psa", bufs=2, space="PSUM") as psa,  # [P,QT]
    ):
        # ---- PE warmup on a gpsimd-zeroed tile (DVE stays free for real
        # work); spans the ~3us p-state ramp while the first DMAs land ----
        wu = const.tile([P, QT], BF16, tag="wu")
        nc.gpsimd.memset(wu[:], 0.0)
        pwu = psk.tile([P, QT], F32, tag="psk", name="pwu")
        for _ in range(12):
            nc.tensor.matmul(pwu[:], lhsT=wu[:, 0:P], rhs=wu[:], start=True,
                             stop=True)

        # ---- DMA order tuned so the K/Q projections can start early ----
        wk_sb = const.tile([P, KO, HD], PD, tag="wk")
        nc.sync.dma_start(wk_sb[:], wk)
        xT_sb = big.tile([P, KO, S], PD, tag="xT")
        nc.sync.dma_start(xT_sb[:, :, 0:QT], xT[:, :, 0:QT])
        wq_sb = const.tile([P, KO, HD], PD, tag="wq")
        nc.sync.dma_start(wq_sb[:], wq)
        bq_sb = const.tile([P, HD // P], F32, tag="bq")
        nc.sync.dma_start(bq_sb[:], bq)
        for c in range(1, QT_N):
            cs = slice(c * QT, (c + 1) * QT)
            nc.sync.dma_start(xT_sb[:, :, cs], xT[:, :, cs])
        wv_sb = const.tile([P, KO, HD], PD, tag="wv")
        nc.sync.dma_start(wv_sb[:], wv)
        wo_sb = const.tile([P, HD // P, D], BF16, tag="wo")
        nc.sync.dma_start(wo_sb[:], wo)

        bias3 = const.tile([P, 1], F32, tag="bias3")
        nc.vector.memset(bias3[:], -3.0)
        # dummy 1-element exp: pulls the ~2.7us ACT table load into the
        # initial DMA wait instead of the first real softmax tile
        dume = const.tile([P, 1], BF16, tag="dume")
        nc.scalar.activation(dume[:], bias3[:], EXP)

        # V with ones column at index DH (-> sumexp row of O'^T)
        v_sb = big.tile([P, KT_N // 2, HPC, 2, VP], AVD, tag="v")
        nc.vector.memset(v_sb[:, :, :, :, DH : DH + 2], 1.0)

        qT_sb = big.tile([P, HD // P, S], BF16, tag="qT")
        kT_sb = big.tile([P, HD // P, S], BF16, tag="kT")
        oT_sb = big.tile([P, HD // P, S], BF16, tag="oT")
        rbs = [const.tile([P, QT], BF16, tag=f"rb{i}", name=f"rb{i}")
               for i in range(2)]
        for t in rbs:
            # rows 65..127 are read (x0) by the pb broadcast matmul; they
            # must not hold NaN bit patterns from uninitialized SBUF
            nc.vector.memset(t[:], 0.0)
        if not USE_GPSIMD_BCAST:
            # selector for the PE partition-broadcast matmul: lhsT row 64
            # (= partition 64, all-ones) x rhs row 64 (1/sumexp) -> pb[0:64]
            sel0 = const.tile([P, DH], BF16, tag="sel0")
            nc.vector.memset(sel0[:], 0.0)
            nc.vector.memset(sel0[DH : DH + 1, :], 1.0)

        def k_proj(mt, c):
            cs = slice(c * QT, (c + 1) * QT)
            pk = psk.tile([P, QT], F32, tag="psk", name="pk")
            _mm_proj(nc, pk[:], wk_sb, xT_sb, slice(mt * P, (mt + 1) * P), cs)
            # bk cancels in softmax; only the 1/WSCALE rescale is applied
            nc.vector.tensor_scalar_mul(kT_sb[:, mt, cs], pk[:], 1.0 / WSCALE)

        def q_proj(qt, mt):
            qs = slice(qt * QT, (qt + 1) * QT)
            pq = psk.tile([P, QT], F32, tag="psk", name="pq")
            _mm_proj(nc, pq[:], wq_sb, xT_sb, slice(mt * P, (mt + 1) * P), qs)
            nc.vector.tensor_scalar(
                qT_sb[:, mt, qs], pq[:], 1.0 / WSCALE, bq_sb[:, mt : mt + 1],
                MUL, ADD,
            )

        def v_proj(st):
            ss = slice(st * P, (st + 1) * P)
            pv = psk.tile([P, QT], F32, tag="psk", name="pv")
            _mm_proj(nc, pv[:, 0:HD], xT_sb, wv_sb, ss, slice(0, HD))
            nc.vector.tensor_scalar_mul(
                v_sb[:, st // 2, :, st % 2, 0:DH],
                pv[:, 0:HD].rearrange("p (h c) -> p h c", h=HPC),
                1.0 / WSCALE,
            )

        def qk_phase(qt, hp, tiles=None, steps=None):
            qs = slice(qt * QT, (qt + 1) * QT)
            dve_set = DVE_SET_EVEN if hp == 0 else DVE_SET_ODD
            if tiles is None:
                tiles = [[None, None] for _ in range(KT_N // 2)]
            for step in steps if steps is not None else range(KT_N // 2):
                kt = 2 * step
                for h2 in range(2):
                    pr = slice(h2 * DH, (h2 + 1) * DH)
                    pqk = psw.tile([P, 2, QT], F32, tag="psw", name="pqk")
                    for j in range(2):
                        nc.tensor.matmul(
                            pqk[:, j, :],
                            lhsT=kT_sb[pr, hp, (kt + j) * P : (kt + j + 1) * P],
                            rhs=qT_sb[pr, hp, qs],
                            start=True,
                            stop=True,
                        )
                    et = exps.tile([P, 2, QT], ETD, tag="exps",
                                   name=f"e{step}{h2}")
                    if (step, h2) in dve_set:
                        nc.vector.tensor_scalar(
                            et[:].bitcast(I16) if not FP8_AV else et[:],
                            pqk[:], A_DVE, B_DVE, MUL, ADD)
                    elif FP8_AV:
                        nc.scalar.activation(
                            et[:].bitcast(FP8), pqk[:], EXP,
                            scale=0.125, bias=bias3[:, 0:1])
                    else:
                        nc.scalar.activation(et[:], pqk[:], EXP,
                                             scale=0.125, bias=bias3[:, 0:1])
                    tiles[step][h2] = et
            return tiles

        def av_phase(qt, hp, tiles, h2_order=(0, 1)):
            qs = slice(qt * QT, (qt + 1) * QT)
            for h2 in h2_order:
                h = 2 * hp + h2
                po = psa.tile([P, QT], F32, tag="psa", name="po")
                if FP8_AV:
                    for t in range(KT_N // 2):
                        nc.tensor.matmul(
                            po[0 : DH + 2, :],
                            lhsT=v_sb[:, t, h, :, 0 : DH + 2],
                            rhs=tiles[t][h2][:].bitcast(FP8),
                            start=(t == 0),
                            stop=(t == KT_N // 2 - 1),
                            perf_mode=DR,
                        )
                else:
                    for kt in range(KT_N):
                        nc.tensor.matmul(
                            po[0 : DH + 1, :],
                            lhsT=v_sb[:, kt // 2, h, kt % 2, 0 : DH + 1],
                            rhs=tiles[kt // 2][h2][:, kt % 2, :],
                            start=(kt == 0),
                            stop=(kt == KT_N - 1),
                        )
                rb = rbs[(2 * qt + hp) % 2]
                with nc.allow_low_precision(
                    reason="bf16 1/sumexp costs ~0.2% rel err, within budget"
                ):
                    nc.vector.reciprocal(rb[DH : DH + 1, :], po[DH : DH + 1, :])
                if USE_GPSIMD_BCAST:
                    bc = work.tile([DH, QT], BF16, tag="bc", name="bc")
                    nc.gpsimd.partition_broadcast(bc[:], rb[DH:P, :])
                    num = po[0:DH, :]
                else:
                    bcp = psa.tile([P, QT], F32, tag="psa", name="bcp")
                    nc.tensor.matmul(bcp[0:DH, :], lhsT=sel0[DH:P, :],
                                     rhs=rb[DH:P, :], start=True, stop=True)
                    bc = bcp[0:DH, :]
                    pon = work.tile([DH, QT], F32, tag="pon", name="pon")
                    nc.vector.tensor_copy(out=pon[:], in_=po[0:DH, :])
                    num = pon[:]
                if h2 == 0:
                    nc.vector.tensor_tensor(
                        oT_sb[0:DH, hp, qs], num, bc, MUL)
                else:
                    # odd head rows belong on partitions 64..127; DVE is
                    # lane-locked, so stage bf16 and lane-shift via DMA
                    sh = work.tile([DH, QT], BF16, tag="sh", name="sh")
                    nc.vector.tensor_tensor(sh[:], num, bc, MUL)
                    nc.sync.dma_start(oT_sb[DH:P, hp, qs], sh[:])

        def o_proj(qt, sts=None, on_dve=False):
            for st in sts if sts is not None else range(QT // P):
                rows = slice(qt * QT + st * P, qt * QT + (st + 1) * P)
                yt = yst.tile([P, D], BF16, tag="yt", name="yt")
                for nt2 in range(D // QT):
                    py = psk.tile([P, QT], F32, tag="psk", name="py")
                    for ks in range(HD // P):
                        nc.tensor.matmul(
                            py[:],
                            lhsT=oT_sb[:, ks, rows],
                            rhs=wo_sb[:, ks, nt2 * QT : (nt2 + 1) * QT],
                            start=(ks == 0),
                            stop=(ks == HD // P - 1),
                        )
                    # staging copy on ACT in steady state (frees DVE for
                    # fast-exp); on DVE for the tail (ACT chews last exps)
                    if on_dve:
                        nc.vector.tensor_copy(
                            out=yt[:, nt2 * QT : (nt2 + 1) * QT], in_=py[:])
                    else:
                        nc.scalar.activation(
                            yt[:, nt2 * QT : (nt2 + 1) * QT], py[:], COPY)
                nc.sync.dma_start(y[rows, :], yt[:])

        # ---- emission schedule (order = scheduler priority) ----
        k_proj(0, 0)
        q_proj(0, 0)
        k_proj(0, 1)
        k_proj(0, 2)
        k_proj(0, 3)
        e = {(0, 0): qk_phase(0, 0)}
        k_proj(1, 0)
        k_proj(1, 1)
        k_proj(1, 2)
        k_proj(1, 3)
        q_proj(0, 1)
        e[(0, 1)] = qk_phase(0, 1)
        for st in range(KT_N):
            v_proj(st)
        q_proj(1, 0)
        q_proj(1, 1)
        for qt in range(1, QT_N):
            lastit = qt == QT_N - 1
            av_phase(qt - 1, 0, e[(qt - 1, 0)], h2_order=(0,))
            t0 = qk_phase(qt, 0, steps=range(0, 4))
            av_phase(qt - 1, 0, e.pop((qt - 1, 0)), h2_order=(1,))
            if lastit:
                t1 = qk_phase(qt, 1, steps=range(0, 4))
            qk_phase(qt, 0, tiles=t0, steps=range(4, 8))
            e[(qt, 0)] = t0
            av_phase(qt - 1, 1, e[(qt - 1, 1)], h2_order=(0,))
            if not lastit:
                t1 = qk_phase(qt, 1, steps=range(0, 4))
            av_phase(qt - 1, 1, e.pop((qt - 1, 1)), h2_order=(1,))
            if lastit:
                qk_phase(qt, 1, tiles=t1, steps=range(4, 8))
            o_proj(qt - 1, sts=(0, 1))
            if not lastit:
                qk_phase(qt, 1, tiles=t1, steps=range(4, 8))
            e[(qt, 1)] = t1
            o_proj(qt - 1, sts=(2, 3))
            if qt + 1 < QT_N:
                q_proj(qt + 1, 0)
                q_proj(qt + 1, 1)
        last = QT_N - 1
        av_phase(last, 0, e.pop((last, 0)))
        # odd head first: its lane-shift DMA (the last o_proj dependency)
        # overlaps the even head's AV + normalize
        av_phase(last, 1, e.pop((last, 1)), h2_order=(1, 0))
        o_proj(last, sts=(0, 1), on_dve=True)
        o_proj(last, sts=(2, 3))


def _prep_inputs(x, Wq, bq, Wk, bk, Wv, bv, Wo, bo):
    bf = ml_dtypes.bfloat16
    pd = ml_dtypes.float8_e4m3 if FP8_PROJ else bf
    x = np.asarray(x, np.float32)
    in_maps = []
    for c in range(N_CORES):
        b, hq = c // 4, c % 4
        cs = slice(hq * HD, (hq + 1) * HD)

        xTr = np.ascontiguousarray(
            x[b].T.reshape(KO, P, S).transpose(1, 0, 2)).astype(pd)

        def wprep(W):
            Wc = np.asarray(W, np.float32)[:, cs] * WSCALE
            return np.ascontiguousarray(
                Wc.reshape(KO, P, HD).transpose(1, 0, 2)).astype(pd)

        in_maps.append({
            "xT": xTr,
            "wq": wprep(Wq),
            "wk": wprep(Wk),
            "wv": wprep(Wv),
            "wo": np.ascontiguousarray(
                np.asarray(Wo, np.float32)[cs, :]
                .reshape(HD // P, P, D).transpose(1, 0, 2)).astype(bf),
            "bq": np.ascontiguousarray(
                np.asarray(bq, np.float32)[cs].reshape(HD // P, P).T),
        })
    return in_maps


def get_program():
    if "nc" not in _CACHE:
        _CACHE["nc"] = _build_program()
    return _CACHE["nc"]


def run(inputs, **kw):
    nc = get_program()
    in_maps = _prep_inputs(**inputs)
    res = run_bass_kernel_spmd(nc, in_maps, core_ids=list(range(N_CORES)), **kw)
    # final bias: bo + bv @ Wo (bv folds out of attention since softmax rows
    # sum to 1), computed in fp32 on host
    bias = np.asarray(inputs["bo"], np.float32) + np.asarray(
        inputs["bv"], np.float32
    ) @ np.asarray(inputs["Wo"], np.float32)
    out = np.empty((2, S, D), np.float32)
    for b in range(2):
        acc = res.results[4 * b]["y"].astype(np.float32)
        for i in range(1, 4):
            acc = acc + res.results[4 * b + i]["y"].astype(np.float32)
        out[b] = acc + bias
    return out, res


def kernel(**inputs):
    out, _ = run(inputs)
    return out
